# revision 35
# baseline (speedup 1.0000x reference)
"""Tensor-parallel GQA attention forward for Trainium2, 8 NeuronCores.

Problem: nn_Attention (B=2, T=2048, D=4096, 32 q heads, 8 kv heads, hd=128).

Sharding (tensor-parallel over heads):
  - core c owns q heads 4c..4c+3 (512 features) and kv head c (128 features)
  - wq/wk/wv column-sharded, wo row-sharded; x replicated (pre-transposed on
    host to x^T [D, B*T] so projections need no on-device transpose)
  - each core returns its partial y @ wo_rows contribution; the host sums the
    8 partials (the unshard step for row-sharded wo).

Matmuls run in bf16 with fp32 PSUM accumulation; softmax sums/reciprocal
stay fp32.

Device dataflow per core:
  P1: q^T/k^T/v^T = W^T x^T (PSUM accum over 32 d-chunks); RoPE fused on
      q^T/k^T via host-permuted even/odd feature order; k^T and v (PE
      transposed) stay resident in SBUF; q^T spills to DRAM scratch.
      Emission starts with k/v of chunks 0-1 so the PE has work while wq
      streams in per-head pieces.
  P2+P3 interleaved, block order (batch, qblock, head): per block,
      scores^T = k^T.T @ q^T with the causal query-range restriction on
      diagonal chunks, one Exp per key-chunk pair (ScalarE), mask, bf16
      pair-sum chain on DVE, ONE ones-matmul per block for the softmax
      denominator, y~^T = v.T @ attn^T (restricted), y = y~^T * recip kept
      in SBUF. After the 4 heads of a (batch, qblock) group finish, the
      wo matmuls for those 512 tokens are emitted (lag-1) so ScalarE exp
      hides under them; partial outputs DMA out as they are produced.
"""

import sys
import types

import numpy as np
import ml_dtypes

BF = ml_dtypes.bfloat16

B = 2
T = 2048
D = 4096
BT = B * T
NH = 32
NKV = 8
HD = 128
N_CORES = 8
QH = NH // N_CORES          # 4 q heads per core
QF = QH * HD                # 512 q features per core
KF = HD                     # 128 kv features per core
TCH = 256                   # phase-1 token chunk
NTC = BT // TCH             # 16 chunks
DC = D // 128               # 32 contraction chunks
QB = 512                    # phase-2 query block
NQB = T // QB               # 4 blocks per (batch, head)
SCALE = 1.0 / float(np.sqrt(HD))


def _install_ntff_hook_shim():
    """antenv.axon_hooks is absent in this image; synthesize it so
    run_bass_kernel_spmd(trace=True) can profile via libaxon_pjrt.so."""
    try:
        from antenv import axon_hooks  # noqa: F401
        return
    except ImportError:
        pass
    try:
        from trn_agent_boot.trn_boot import _ntff_profile_via_ctypes
        hook = _ntff_profile_via_ctypes("/opt/axon/libaxon_pjrt.so")
    except Exception:
        hook = None
    mod = types.ModuleType("antenv.axon_hooks")
    mod._hook = hook
    mod.get_axon_ntff_profile_hook = lambda: mod._hook

    def _set(h):
        mod._hook = h

    mod.set_axon_hooks = _set
    mod.set_axon_ntff_profile_hook = _set
    sys.modules["antenv.axon_hooks"] = mod


_install_ntff_hook_shim()

import concourse.bass as bass  # noqa: E402,F401
import concourse.bacc as bacc  # noqa: E402
import concourse.tile as tile  # noqa: E402
import concourse.mybir as mybir  # noqa: E402
from concourse import bass_utils  # noqa: E402
from concourse.masks import make_identity  # noqa: E402

F32 = mybir.dt.float32
BF16 = mybir.dt.bfloat16
EXP = mybir.ActivationFunctionType.Exp

_NC_CACHE = []


def build():
    nc = bacc.Bacc("TRN2", target_bir_lowering=False, debug=False,
                   num_devices=N_CORES)

    xT = nc.dram_tensor("xT", [128, NTC, DC, TCH], BF16, kind="ExternalInput").ap()
    wq = nc.dram_tensor("wq", [QH, 128, DC, 128], BF16, kind="ExternalInput").ap()
    wk = nc.dram_tensor("wk", [128, DC, KF], BF16, kind="ExternalInput").ap()
    wv = nc.dram_tensor("wv", [128, DC, KF], BF16, kind="ExternalInput").ap()
    wo = nc.dram_tensor("wo", [128, QH, D], BF16, kind="ExternalInput").ap()
    cc = nc.dram_tensor("cc", [128, T], BF16, kind="ExternalInput").ap()
    ss = nc.dram_tensor("ss", [128, T], BF16, kind="ExternalInput").ap()
    out = nc.dram_tensor("out", [BT, D], BF16, kind="ExternalOutput").ap()

    with tile.TileContext(nc) as tc:
        _build_body(nc, tc, xT, wq, wk, wv, wo, cc, ss, out)
    nc.compile()
    return nc


def _build_body(nc, tc, xT, wq, wk, wv, wo, cc, ss, out):
    dram = tc.alloc_tile_pool(name="dram", bufs=1, space="DRAM")
    const = tc.alloc_tile_pool(name="const", bufs=1)
    # right-side pools live across the whole kernel (no aliasing with the
    # released phase-1 pools -> their DMAs never gate on phase-1 matmuls)
    wopool = tc.alloc_tile_pool(name="wo", bufs=1, side="right")
    kvpool = tc.alloc_tile_pool(name="kv", bufs=1, side="right")
    ypool = tc.alloc_tile_pool(name="y", bufs=2, side="right")
    qpool = tc.alloc_tile_pool(name="q", bufs=3, side="right")
    # PSUM: phase 1 uses ps_proj (4 banks) + ps_t (transposes); phase 2/3
    # use ps2 (2x2 banks) + ps_one ring (4 single banks).
    ps_proj = tc.alloc_tile_pool(name="ps_proj", bufs=4, space="PSUM")
    ps_t = tc.alloc_tile_pool(name="ps_t", bufs=2, space="PSUM")
    # phase-1 pools (released after phase 1 is emitted)
    wpool = tc.alloc_tile_pool(name="weights", bufs=1)
    xpool = tc.alloc_tile_pool(name="xstream", bufs=3)
    cspool = tc.alloc_tile_pool(name="cs", bufs=1)
    rpool = tc.alloc_tile_pool(name="rope", bufs=4)

    # ---- DRAM scratch: only q^T spills (k/v/y stay in SBUF) ----
    qT_s = [dram.tile([QH, 128, T], BF16, tag=f"qTs{b}", name=f"qTs{b}")
            for b in range(B)]

    wo_sb = wopool.tile([128, QH, D], BF16)
    # k^T [d, tok] and v [tok->partition, d] resident for both batches
    kts_sb = [kvpool.tile([128, T], BF16, tag=f"kts{b}", name=f"kts{b}")
              for b in range(B)]
    vts_sb = [kvpool.tile([128, T // 128, 128], BF16, tag=f"vts{b}",
                          name=f"vts{b}") for b in range(B)]

    # ---- startup DMAs (before any compute emission: nothing may sit in
    # front of these on the queue sequencers) ----
    # Each dma_start costs ~0.7us of queue issue time, so splits are chosen
    # to balance issue-rate cost against dependency granularity.
    # DMA arbitration round-robins per descriptor, so transfers with small
    # per-partition rows get starved by large-row ones: fetch whole x
    # chunks in ONE DMA (16KB contiguous rows). Chunk 0 uses 4 tile-DMAs
    # (4KB rows) so the first matmuls can start before the full chunk lands.
    def fetch_x(t, split=False):
        npc = DC // 4
        if split:
            xts = []
            for sx in range(4):
                xst = xpool.tile([128, npc, TCH], BF16, tag=f"xt{sx}",
                                 name=f"xt{sx}", bufs=1)
                nc.sync.dma_start(xst[:],
                                  xT[:, t, sx * npc:(sx + 1) * npc, :])
                xts.append(xst)
            return xts
        xbig = xpool.tile([128, DC, TCH], BF16, tag="xbig", name="xbig",
                          bufs=2)
        nc.sync.dma_start(xbig[:], xT[:, t])
        return [xbig[:, sx * npc:(sx + 1) * npc, :] for sx in range(4)]

    # x0 in two 1MB halves (8KB rows): first k-matmuls start when the first
    # half lands instead of waiting for the whole 2MB chunk
    x0 = xpool.tile([128, DC, TCH], BF16, tag="xbig", name="xbig", bufs=2)
    npc0 = DC // 2
    for hf in range(2):
        nc.sync.dma_start(x0[:, hf * npc0:(hf + 1) * npc0, :],
                          xT[:, 0, hf * npc0:(hf + 1) * npc0, :])
    cur_x = [x0[:, sx * (DC // 4):(sx + 1) * (DC // 4), :] for sx in range(4)]

    wk_sb = wpool.tile([128, DC, KF], BF16, name="wk_sb")
    nc.scalar.dma_start(wk_sb[:], wk[:])
    wv_sb = wpool.tile([128, DC, KF], BF16, name="wv_sb")
    nc.scalar.dma_start(wv_sb[:], wv[:])
    # cos/sin tables come in as bf16 (halves startup bytes) on the otherwise
    # idle gpsimd queue, then upconvert once to fp32 on-device
    cc_bf = cspool.tile([128, T], BF16, name="cc_bf")
    ss_bf = cspool.tile([128, T], BF16, name="ss_bf")
    nc.gpsimd.dma_start(cc_bf[:], cc[:])
    nc.gpsimd.dma_start(ss_bf[:], ss[:])
    # wq per-head pieces split across the scalar queue (behind wk/wv) and
    # the gpsimd queue (behind cos/sin) so the head pieces land in fc order
    wq_sb = []
    for fc in range(QH):
        wt = wpool.tile([128, DC, 128], BF16, tag=f"wq{fc}", name=f"wq{fc}")
        eng = nc.scalar if fc < 2 else nc.gpsimd
        eng.dma_start(wt[:], wq[fc])
        wq_sb.append(wt)

    next_x = fetch_x(1)

    # ---- constants (emitted after the startup DMA issues) ----
    ident = const.tile([128, 128], F32)
    make_identity(nc, ident[:])
    ident_bf = const.tile([128, 128], BF16)
    nc.vector.tensor_copy(ident_bf[:], ident[:])
    onesPP = const.tile([128, 128], BF16)
    nc.vector.memset(onesPP[:], 1.0)
    # preload the exp activation table (after the weight DMA issues: the
    # ~2.7us ACT_TABLE_LOAD must not block them on the scalar queue)
    dummy = const.tile([128, 1], BF16)
    nc.scalar.activation(dummy[:], ident[:, 0:1], EXP)
    # BIG[p, v] = 1.0 iff v - 384 >= p ; mask(delta) = BIG[:, 384-delta :][:QB]
    BIGf = const.tile([128, 896], F32)
    nc.gpsimd.memset(BIGf[:], 1.0)
    nc.gpsimd.affine_select(
        out=BIGf[:], in_=BIGf[:], compare_op=mybir.AluOpType.is_ge,
        fill=0.0, base=-384, channel_multiplier=-1, pattern=[[1, 896]],
    )
    BIG = const.tile([128, 896], BF16)
    nc.vector.tensor_copy(BIG[:], BIGf[:])

    cc_sb = cspool.tile([128, T], F32, name="cc_sb")
    ss_sb = cspool.tile([128, T], F32, name="ss_sb")
    nc.vector.tensor_copy(cc_sb[:], cc_bf[:])
    nc.vector.tensor_copy(ss_sb[:], ss_bf[:])

    def rope_evict(ps, tsl, dst_ap, direct=False):
        """psum [128, TCH] -> RoPE (fp32) -> bf16 -> dst (SBUF or DRAM)."""
        raw = rpool.tile([128, TCH], F32, tag="rraw")
        nc.any.tensor_copy(raw[:], ps[:])
        swp = rpool.tile([128, TCH], F32, tag="rswp")
        nc.vector.tensor_copy(swp[0:64, :], raw[64:128, :])
        nc.vector.tensor_copy(swp[64:128, :], raw[0:64, :])
        nc.vector.tensor_mul(out=swp[:], in0=swp[:], in1=ss_sb[:, tsl])
        t1 = rpool.tile([128, TCH], BF16, tag="rt1")
        nc.vector.tensor_mul(out=t1[:], in0=raw[:], in1=cc_sb[:, tsl])
        if direct:
            nc.vector.tensor_add(out=dst_ap, in0=t1[:], in1=swp[:])
        else:
            nc.vector.tensor_add(out=t1[:], in0=t1[:], in1=swp[:])
            nc.sync.dma_start(dst_ap, t1[:])

    def kv_proj(t, xts):
        b, tloc = divmod(t * TCH, T)
        lsl = slice(tloc, tloc + TCH)
        ps = ps_proj.tile([128, 512], F32, tag="big", name="ps")[:, :TCH]
        for dc in range(DC):
            nc.tensor.matmul(ps[:], wk_sb[:, dc, :], xts[dc // 8][:, dc % 8, :],
                             start=(dc == 0), stop=(dc == DC - 1))
        rope_evict(ps, lsl, kts_sb[b][:, lsl], direct=True)

        ps = ps_proj.tile([128, 512], F32, tag="big", name="ps")[:, :TCH]
        for dc in range(DC):
            nc.tensor.matmul(ps[:], wv_sb[:, dc, :], xts[dc // 8][:, dc % 8, :],
                             start=(dc == 0), stop=(dc == DC - 1))
        vraw = rpool.tile([128, TCH], BF16, tag="vraw")
        nc.any.tensor_copy(vraw[:], ps[:])
        for j in range(TCH // 128):
            pst = ps_t.tile([128, 128], BF16, tag="pst", name="pst")
            nc.tensor.transpose(pst[:], vraw[:, j * 128:(j + 1) * 128],
                                ident_bf[:])
            g = (tloc // 128) + j
            nc.any.tensor_copy(vts_sb[b][:, g, :], pst[:])

    def q_proj(t, xts):
        b, tloc = divmod(t * TCH, T)
        lsl = slice(tloc, tloc + TCH)
        for fc in range(QH):
            ps = ps_proj.tile([128, 512], F32, tag="big", name="ps")[:, :TCH]
            for dc in range(DC):
                nc.tensor.matmul(
                    ps[:], wq_sb[fc][:, dc, :], xts[dc // 8][:, dc % 8, :],
                    start=(dc == 0), stop=(dc == DC - 1))
            rope_evict(ps, lsl, qT_s[b][fc][:, lsl])

    # phase-2 block order and qT fetch, defined here so the first two qT
    # blocks can be prefetched from inside the phase-1 loop (their DMA
    # issues then sit early in the scalar queue instead of behind all of
    # phase 1)
    blocks = [(b, qb, h) for b in range(B) for qb in range(NQB)
              for h in range(QH)]

    def fetch_qT(i):
        b, qb, h = blocks[i]
        qT_sb = qpool.tile([128, QB], BF16, tag="qT", name="qT_sb")
        nc.scalar.dma_start(qT_sb[:], qT_s[b][h][:, qb * QB:(qb + 1) * QB])
        return qT_sb

    qT_ring = []

    kv_proj(0, cur_x)
    q_proj(0, cur_x)
    cur_x = next_x
    for t in range(1, NTC):
        xts = cur_x
        if t + 1 < NTC:
            cur_x = fetch_x(t + 1)
        kv_proj(t, xts)
        q_proj(t, xts)
        if t == 4:
            # wo preload on the (otherwise idle) gpsimd SWDGE queue; delayed
            # past startup so it doesn't steal DMA bandwidth from x/weights
            for fc in range(QH):
                nc.gpsimd.dma_start(wo_sb[:, fc, :], wo[:, fc, :])
        if t == NTC - 3:
            qT_ring.append(fetch_qT(0))
            qT_ring.append(fetch_qT(1))

    rpool.release()
    cspool.release()
    xpool.release()
    wpool.release()
    ps_t.release()
    ps_proj.release()

    apool = tc.alloc_tile_pool(name="attn", bufs=4)
    opool = tc.alloc_tile_pool(name="outev", bufs=6)
    # PSUM rings (8 banks total): scores 2x[128,2,512] (4 banks), attnV
    # accumulators 2x (2 banks), denominator/wo-accumulator shared ring 2x
    # (2 banks). Keeping ps_yt (block-lived) OUT of the fast-cycling rings
    # avoids phase-3 matmuls landing on a bank still held by a live
    # attention accumulator.
    ps2pool = tc.alloc_tile_pool(name="ps2", bufs=2, space="PSUM")
    psy = tc.alloc_tile_pool(name="psy", bufs=2, space="PSUM")
    psm = tc.alloc_tile_pool(name="psm", bufs=2, space="PSUM")

    # ---- phase 2+3 interleaved ----
    # block order (b, qb, h): after the 4 heads of (b, qb) finish, the wo
    # matmuls for those 512 tokens are emitted (lag-1 behind the next
    # block's first score pair).
    pending = []     # deferred consume/tail thunks (lag-1 across blocks)
    pending_p3 = []  # deferred wo-matmul groups (drained one pair later so
                     # the previous group's DVE tail has PE work to hide
                     # behind before the wo matmuls need its yt tiles)
    yt_group = {}    # (b, qb) -> [yt tiles by h]

    def drain_pending():
        while pending:
            pending.pop(0)()

    def drain_pending_p3(n=None):
        cnt = 0
        while pending_p3 and (n is None or cnt < n):
            pending_p3.pop(0)()
            cnt += 1

    def phase3_tcl(b, tg, tcl, yts, last=False):
        row0 = b * T + tg * 512 + tcl * 128
        if True:
            # one full-row [128, 4096] out tile per 128 tokens: the single
            # 8KB-row DMA wins arbitration and issue-rate over 4 small ones
            ot = opool.tile([128, D], BF16, tag="ot", name="ot")
            for oc in range(D // 512):
                ps = psm.tile([128, 512], F32, tag="sm", name="pso")
                for fc in range(QH):
                    nc.tensor.matmul(
                        ps[:],
                        yts[fc][:, tcl * 128:(tcl + 1) * 128],
                        wo_sb[:, fc, oc * 512:(oc + 1) * 512],
                        start=(fc == 0), stop=(fc == QH - 1))
                # explicit engine alternation: nc.any tends to dump all
                # copies on ScalarE, whose FIFO then blocks exp
                if oc % 2 == 0:
                    nc.vector.tensor_copy(ot[:, oc * 512:(oc + 1) * 512],
                                          ps[:])
                else:
                    nc.scalar.copy(ot[:, oc * 512:(oc + 1) * 512], ps[:])
            nsplit = 4 if (last and tcl == 3) else 1
            for p in range(nsplit):
                r0 = row0 + p * 128 // nsplit
                r1 = row0 + (p + 1) * 128 // nsplit
                nc.sync.dma_start(out[r0:r1, :], ot[r0 - row0:r1 - row0, :])

    for i, (b, qb, h) in enumerate(blocks):
        nkc = 4 * (qb + 1)
        npair = nkc // 2
        qT_sb = qT_ring.pop(0)
        if i + 2 < len(blocks):
            qT_ring.append(fetch_qT(i + 2))

        ps_yt = psy.tile([128, 512], F32, tag="yt", name="ps_yt")
        yt = ypool.tile([128, QB], BF16, tag=f"yt{h}", name=f"yt{h}")
        yt_group.setdefault((b, qb), [None] * QH)[h] = yt
        a2s = []
        acc = [None]

        def consume(ip, nkc=nkc, qb=qb, ps_yt=ps_yt, a2s=a2s, vts=vts_sb[b]):
            a2 = a2s[ip]
            for j in range(2):
                c = 2 * ip + j
                qs = max(0, (c - 4 * qb) * 128)
                nc.tensor.matmul(ps_yt[:, qs:], vts[:, c, :], a2[:, j, qs:],
                                 start=(c == 0), stop=(c == nkc - 1))

        def ones_tail(b=b, qb=qb, h=h, ps_yt=ps_yt, acc=acc, yt=yt):
            ps_bc = psm.tile([128, 512], F32, tag="sm", name="ps_bc")
            nc.tensor.matmul(ps_bc[:], onesPP[:], acc[0][:],
                             start=True, stop=True)
            rb = qpool.tile([128, QB], F32, tag="rb", name="rb")
            nc.vector.reciprocal_approx_fast(out=rb[:], in_=ps_bc[:])
            nc.vector.tensor_mul(out=yt[:], in0=ps_yt[:], in1=rb[:])

        for ip in range(npair):
            # qsp: first query column this PAIR can influence; the score
            # matmul/exp/mask/pair-sum all restrict to [qsp:] so exp never
            # reads unwritten PSUM (masked-out garbage within [qsp:] is
            # zeroed by the BIG mask as in the unrestricted scheme)
            qsp = max(0, (2 * ip - 4 * qb) * 128)
            ps2 = ps2pool.tile([128, 2, 512], F32, tag="s", name="ps2")
            for j in range(2):
                c = 2 * ip + j
                nc.tensor.matmul(ps2[:, j, qsp:], kts_sb[b][:, c * 128:(c + 1) * 128],
                                 qT_sb[:, qsp:], start=True, stop=True)
            if ip == 0:
                # previous block's deferred tail runs behind our first
                # score pair, so its exp wait never idles the PE
                drain_pending()
            elif ip == 1:
                # up to 2-3 wo-matmul tcl-slices (32 MMs each) per block:
                # spreading them across the group's blocks gives every
                # block a PE reservoir that hides the exp latency chain
                # (3 in the small qb==0 blocks keeps the ledger balanced:
                # 8x3 + 20x2 + final 4 = 16 groups x 4 slices)
                drain_pending_p3(3 if qb == 0 else 2)
            a2 = apool.tile([128, 2, 512], BF16, tag="a", name="a2")
            nc.scalar.activation(a2[:, :, qsp:], ps2[:, :, qsp:], EXP,
                                 scale=SCALE)
            for j in range(2):
                c = 2 * ip + j
                delta = c * 128 - qb * QB
                if delta >= 0:
                    off = 384 - delta
                    nc.vector.tensor_mul(
                        out=a2[:, j, qsp:], in0=a2[:, j, qsp:],
                        in1=BIG[:, off + qsp:off + QB])
            if ip == 0:
                acc[0] = apool.tile([128, 512], BF16, tag="acc", name="acc")
                nc.vector.tensor_add(out=acc[0][:], in0=a2[:, 0, :],
                                     in1=a2[:, 1, :])
            else:
                tmp = apool.tile([128, 512], BF16, tag="tmp", name="tmp")
                nc.vector.tensor_add(out=tmp[:, qsp:], in0=a2[:, 0, qsp:],
                                     in1=a2[:, 1, qsp:])
                nc.vector.tensor_add(out=acc[0][:, qsp:],
                                     in0=acc[0][:, qsp:],
                                     in1=tmp[:, qsp:])
            a2s.append(a2)
            if ip >= 2:
                consume(ip - 2)
        if npair >= 2:
            pending.append(lambda c=consume, n=npair: c(n - 2))
        pending.append(lambda c=consume, n=npair: c(n - 1))
        pending.append(ones_tail)
        if h == QH - 1:
            last = (i == len(blocks) - 1)
            yts = yt_group.pop((b, qb))
            for tcl in range(4):
                pending_p3.append(
                    lambda b=b, qb=qb, tcl=tcl, yts=yts, last=last:
                    phase3_tcl(b, qb, tcl, yts, last=last))
            if last:
                drain_pending()
                drain_pending_p3()
    drain_pending()
    drain_pending_p3()

    opool.release()
    apool.release()
    psm.release()
    psy.release()
    ps2pool.release()
    qpool.release()
    ypool.release()
    kvpool.release()
    wopool.release()
    const.release()
    dram.release()


_PERM = np.concatenate([np.arange(0, HD, 2), np.arange(1, HD, 2)])


def _prep_inputs(x, freqs_cis, wq, wk, wv, wo):
    x = np.asarray(x, dtype=np.float32)
    freqs_cis = np.asarray(freqs_cis, dtype=np.float32)
    wq = np.asarray(wq, dtype=np.float32)
    wk = np.asarray(wk, dtype=np.float32)
    wv = np.asarray(wv, dtype=np.float32)
    wo = np.asarray(wo, dtype=np.float32)

    x2 = x.reshape(BT, D)
    # [di, tchunk, dc, tlocal] so each phase-1 chunk DMA is 128 x 32KB contig
    xTq = np.ascontiguousarray(
        x2.reshape(NTC, TCH, DC, 128).transpose(3, 0, 2, 1)).astype(BF)

    cosv = freqs_cis[:, :, 0].T                      # [64, T]
    sinv = freqs_cis[:, :, 1].T
    cc = np.ascontiguousarray(
        np.concatenate([cosv, cosv], axis=0)).astype(BF)   # [128, T]
    ss = np.ascontiguousarray(
        np.concatenate([-sinv, sinv], axis=0)).astype(BF)

    in_maps = []
    for c in range(N_CORES):
        wq_fc = np.stack([
            np.ascontiguousarray(
                wq[:, (4 * c + fc) * HD + _PERM]
                .reshape(DC, 128, 128).transpose(1, 0, 2))
            for fc in range(QH)])
        kcols = c * HD + _PERM
        in_maps.append({
            "xT": xTq,
            "wq": wq_fc.astype(BF),
            "wk": np.ascontiguousarray(
                wk[:, kcols].reshape(DC, 128, KF).transpose(1, 0, 2))
                .astype(BF),
            "wv": np.ascontiguousarray(
                wv[:, c * HD:(c + 1) * HD].reshape(DC, 128, KF)
                .transpose(1, 0, 2)).astype(BF),
            "wo": np.ascontiguousarray(
                wo[c * QF:(c + 1) * QF, :].reshape(QH, 128, D)
                .transpose(1, 0, 2)).astype(BF),
            "cc": cc,
            "ss": ss,
        })
    return in_maps


def kernel(x, freqs_cis, wq, wk, wv, wo):
    if not _NC_CACHE:
        _NC_CACHE.append(build())
    nc = _NC_CACHE[0]
    in_maps = _prep_inputs(x, freqs_cis, wq, wk, wv, wo)
    res = None
    err = None
    for _attempt in range(3):
        try:
            res = bass_utils.run_bass_kernel_spmd(
                nc, in_maps, core_ids=list(range(N_CORES)))
            break
        except Exception as e:  # transient NRT device wedge: retry
            err = e
            import time as _time
            _time.sleep(5)
    if res is None:
        raise err
    acc = res.results[0]["out"].astype(np.float32)
    for i in range(1, N_CORES):
        acc += res.results[i]["out"].astype(np.float32)
    return acc.reshape(B, T, D)


if __name__ == "__main__":
    rng = np.random.default_rng(0)
    s = 1.0 / np.sqrt(D)
    inputs = {
        "x": rng.standard_normal((B, T, D), dtype=np.float32),
        "freqs_cis": rng.standard_normal((T, HD // 2, 2), dtype=np.float32),
        "wq": rng.standard_normal((D, NH * HD), dtype=np.float32) * s,
        "wk": rng.standard_normal((D, NKV * HD), dtype=np.float32) * s,
        "wv": rng.standard_normal((D, NKV * HD), dtype=np.float32) * s,
        "wo": rng.standard_normal((D, D), dtype=np.float32) * s,
    }
    out = kernel(**inputs)
    print("out", out.shape, out.dtype, float(np.abs(out).mean()))


# revision 36
# speedup vs baseline: 1.0126x; 1.0126x over previous
"""Tensor-parallel GQA attention forward for Trainium2, 8 NeuronCores.

Problem: nn_Attention (B=2, T=2048, D=4096, 32 q heads, 8 kv heads, hd=128).

Sharding (tensor-parallel over heads):
  - core c owns q heads 4c..4c+3 (512 features) and kv head c (128 features)
  - wq/wk/wv column-sharded, wo row-sharded; x replicated (pre-transposed on
    host to x^T [D, B*T] so projections need no on-device transpose)
  - each core returns its partial y @ wo_rows contribution; the host sums the
    8 partials (the unshard step for row-sharded wo).

Matmuls run in bf16 with fp32 PSUM accumulation; softmax sums/reciprocal
stay fp32.

Device dataflow per core:
  P1: q^T/k^T/v^T = W^T x^T (PSUM accum over 32 d-chunks); RoPE fused on
      q^T/k^T via host-permuted even/odd feature order; k^T and v (PE
      transposed) stay resident in SBUF; q^T spills to DRAM scratch.
      Emission starts with k/v of chunks 0-1 so the PE has work while wq
      streams in per-head pieces.
  P2+P3 interleaved, block order (batch, qblock, head): per block,
      scores^T = k^T.T @ q^T with the causal query-range restriction on
      diagonal chunks, one Exp per key-chunk pair (ScalarE), mask, bf16
      pair-sum chain on DVE, ONE ones-matmul per block for the softmax
      denominator, y~^T = v.T @ attn^T (restricted), y = y~^T * recip kept
      in SBUF. After the 4 heads of a (batch, qblock) group finish, the
      wo matmuls for those 512 tokens are emitted (lag-1) so ScalarE exp
      hides under them; partial outputs DMA out as they are produced.
"""

import sys
import types

import numpy as np
import ml_dtypes

BF = ml_dtypes.bfloat16

B = 2
T = 2048
D = 4096
BT = B * T
NH = 32
NKV = 8
HD = 128
N_CORES = 8
QH = NH // N_CORES          # 4 q heads per core
QF = QH * HD                # 512 q features per core
KF = HD                     # 128 kv features per core
TCH = 256                   # phase-1 token chunk
NTC = BT // TCH             # 16 chunks
DC = D // 128               # 32 contraction chunks
QB = 512                    # phase-2 query block
NQB = T // QB               # 4 blocks per (batch, head)
SCALE = 1.0 / float(np.sqrt(HD))


def _install_ntff_hook_shim():
    """antenv.axon_hooks is absent in this image; synthesize it so
    run_bass_kernel_spmd(trace=True) can profile via libaxon_pjrt.so."""
    try:
        from antenv import axon_hooks  # noqa: F401
        return
    except ImportError:
        pass
    try:
        from trn_agent_boot.trn_boot import _ntff_profile_via_ctypes
        hook = _ntff_profile_via_ctypes("/opt/axon/libaxon_pjrt.so")
    except Exception:
        hook = None
    mod = types.ModuleType("antenv.axon_hooks")
    mod._hook = hook
    mod.get_axon_ntff_profile_hook = lambda: mod._hook

    def _set(h):
        mod._hook = h

    mod.set_axon_hooks = _set
    mod.set_axon_ntff_profile_hook = _set
    sys.modules["antenv.axon_hooks"] = mod


_install_ntff_hook_shim()

import concourse.bass as bass  # noqa: E402,F401
import concourse.bacc as bacc  # noqa: E402
import concourse.tile as tile  # noqa: E402
import concourse.mybir as mybir  # noqa: E402
from concourse import bass_utils  # noqa: E402
from concourse.masks import make_identity  # noqa: E402

F32 = mybir.dt.float32
BF16 = mybir.dt.bfloat16
EXP = mybir.ActivationFunctionType.Exp

_NC_CACHE = []


def build():
    nc = bacc.Bacc("TRN2", target_bir_lowering=False, debug=False,
                   num_devices=N_CORES)

    xT = nc.dram_tensor("xT", [128, NTC, DC, TCH], BF16, kind="ExternalInput").ap()
    wq = nc.dram_tensor("wq", [QH, 128, DC, 128], BF16, kind="ExternalInput").ap()
    wk = nc.dram_tensor("wk", [128, DC, KF], BF16, kind="ExternalInput").ap()
    wv = nc.dram_tensor("wv", [128, DC, KF], BF16, kind="ExternalInput").ap()
    wo = nc.dram_tensor("wo", [128, QH, D], BF16, kind="ExternalInput").ap()
    cc = nc.dram_tensor("cc", [128, T], BF16, kind="ExternalInput").ap()
    ss = nc.dram_tensor("ss", [128, T], BF16, kind="ExternalInput").ap()
    out = nc.dram_tensor("out", [BT, D], BF16, kind="ExternalOutput").ap()

    with tile.TileContext(nc) as tc:
        _build_body(nc, tc, xT, wq, wk, wv, wo, cc, ss, out)
    nc.compile()
    return nc


def _build_body(nc, tc, xT, wq, wk, wv, wo, cc, ss, out):
    dram = tc.alloc_tile_pool(name="dram", bufs=1, space="DRAM")
    const = tc.alloc_tile_pool(name="const", bufs=1)
    # right-side pools live across the whole kernel (no aliasing with the
    # released phase-1 pools -> their DMAs never gate on phase-1 matmuls)
    wopool = tc.alloc_tile_pool(name="wo", bufs=1, side="right")
    kvpool = tc.alloc_tile_pool(name="kv", bufs=1, side="right")
    ypool = tc.alloc_tile_pool(name="y", bufs=2, side="right")
    qpool = tc.alloc_tile_pool(name="q", bufs=3, side="right")
    # PSUM: phase 1 uses ps_proj (4 banks) + ps_t (transposes); phase 2/3
    # use ps2 (2x2 banks) + ps_one ring (4 single banks).
    ps_proj = tc.alloc_tile_pool(name="ps_proj", bufs=4, space="PSUM")
    ps_t = tc.alloc_tile_pool(name="ps_t", bufs=2, space="PSUM")
    # phase-1 pools (released after phase 1 is emitted)
    wpool = tc.alloc_tile_pool(name="weights", bufs=1)
    xpool = tc.alloc_tile_pool(name="xstream", bufs=3)
    cspool = tc.alloc_tile_pool(name="cs", bufs=1)
    rpool = tc.alloc_tile_pool(name="rope", bufs=4)

    # ---- DRAM scratch: only q^T spills (k/v/y stay in SBUF) ----
    qT_s = [dram.tile([QH, 128, T], BF16, tag=f"qTs{b}", name=f"qTs{b}")
            for b in range(B)]

    wo_sb = wopool.tile([128, QH, D], BF16)
    # k^T [d, tok] and v [tok->partition, d] resident for both batches
    kts_sb = [kvpool.tile([128, T], BF16, tag=f"kts{b}", name=f"kts{b}")
              for b in range(B)]
    vts_sb = [kvpool.tile([128, T // 128, 128], BF16, tag=f"vts{b}",
                          name=f"vts{b}") for b in range(B)]

    # ---- startup DMAs (before any compute emission: nothing may sit in
    # front of these on the queue sequencers) ----
    # Each dma_start costs ~0.7us of queue issue time, so splits are chosen
    # to balance issue-rate cost against dependency granularity.
    # DMA arbitration round-robins per descriptor, so transfers with small
    # per-partition rows get starved by large-row ones: fetch whole x
    # chunks in ONE DMA (16KB contiguous rows). Chunk 0 uses 4 tile-DMAs
    # (4KB rows) so the first matmuls can start before the full chunk lands.
    def fetch_x(t, split=False):
        npc = DC // 4
        if split:
            xts = []
            for sx in range(4):
                xst = xpool.tile([128, npc, TCH], BF16, tag=f"xt{sx}",
                                 name=f"xt{sx}", bufs=1)
                nc.sync.dma_start(xst[:],
                                  xT[:, t, sx * npc:(sx + 1) * npc, :])
                xts.append(xst)
            return xts
        xbig = xpool.tile([128, DC, TCH], BF16, tag="xbig", name="xbig",
                          bufs=2)
        nc.sync.dma_start(xbig[:], xT[:, t])
        return [xbig[:, sx * npc:(sx + 1) * npc, :] for sx in range(4)]

    # x0 in two 1MB halves (8KB rows): first k-matmuls start when the first
    # half lands instead of waiting for the whole 2MB chunk
    x0 = xpool.tile([128, DC, TCH], BF16, tag="xbig", name="xbig", bufs=2)
    npc0 = DC // 2
    for hf in range(2):
        nc.sync.dma_start(x0[:, hf * npc0:(hf + 1) * npc0, :],
                          xT[:, 0, hf * npc0:(hf + 1) * npc0, :])
    cur_x = [x0[:, sx * (DC // 4):(sx + 1) * (DC // 4), :] for sx in range(4)]

    wk_sb = wpool.tile([128, DC, KF], BF16, name="wk_sb")
    nc.scalar.dma_start(wk_sb[:], wk[:])
    wv_sb = wpool.tile([128, DC, KF], BF16, name="wv_sb")
    nc.scalar.dma_start(wv_sb[:], wv[:])
    # cos/sin tables come in as bf16 (halves startup bytes) on the otherwise
    # idle gpsimd queue, then upconvert once to fp32 on-device
    cc_bf = cspool.tile([128, T], BF16, name="cc_bf")
    ss_bf = cspool.tile([128, T], BF16, name="ss_bf")
    nc.gpsimd.dma_start(cc_bf[:], cc[:])
    nc.gpsimd.dma_start(ss_bf[:], ss[:])
    # wq per-head pieces split across the scalar queue (behind wk/wv) and
    # the gpsimd queue (behind cos/sin) so the head pieces land in fc order
    wq_sb = []
    for fc in range(QH):
        wt = wpool.tile([128, DC, 128], BF16, tag=f"wq{fc}", name=f"wq{fc}")
        eng = nc.scalar if fc < 2 else nc.gpsimd
        eng.dma_start(wt[:], wq[fc])
        wq_sb.append(wt)

    next_x = fetch_x(1)

    # ---- constants (emitted after the startup DMA issues) ----
    ident = const.tile([128, 128], F32)
    make_identity(nc, ident[:])
    ident_bf = const.tile([128, 128], BF16)
    nc.vector.tensor_copy(ident_bf[:], ident[:])
    onesPP = const.tile([128, 128], BF16)
    nc.vector.memset(onesPP[:], 1.0)
    # preload the exp activation table (after the weight DMA issues: the
    # ~2.7us ACT_TABLE_LOAD must not block them on the scalar queue)
    dummy = const.tile([128, 1], BF16)
    nc.scalar.activation(dummy[:], ident[:, 0:1], EXP)
    # BIG[p, v] = 1.0 iff v - 384 >= p ; mask(delta) = BIG[:, 384-delta :][:QB]
    BIGf = const.tile([128, 896], F32)
    nc.gpsimd.memset(BIGf[:], 1.0)
    nc.gpsimd.affine_select(
        out=BIGf[:], in_=BIGf[:], compare_op=mybir.AluOpType.is_ge,
        fill=0.0, base=-384, channel_multiplier=-1, pattern=[[1, 896]],
    )
    BIG = const.tile([128, 896], BF16)
    nc.vector.tensor_copy(BIG[:], BIGf[:])

    cc_sb = cspool.tile([128, T], F32, name="cc_sb")
    ss_sb = cspool.tile([128, T], F32, name="ss_sb")
    nc.vector.tensor_copy(cc_sb[:], cc_bf[:])
    nc.vector.tensor_copy(ss_sb[:], ss_bf[:])

    def rope_evict(ps, tsl, dst_ap, direct=False):
        """psum [128, TCH] -> RoPE (fp32) -> bf16 -> dst (SBUF or DRAM)."""
        raw = rpool.tile([128, TCH], F32, tag="rraw")
        nc.any.tensor_copy(raw[:], ps[:])
        swp = rpool.tile([128, TCH], F32, tag="rswp")
        nc.vector.tensor_copy(swp[0:64, :], raw[64:128, :])
        nc.vector.tensor_copy(swp[64:128, :], raw[0:64, :])
        nc.vector.tensor_mul(out=swp[:], in0=swp[:], in1=ss_sb[:, tsl])
        t1 = rpool.tile([128, TCH], BF16, tag="rt1")
        nc.vector.tensor_mul(out=t1[:], in0=raw[:], in1=cc_sb[:, tsl])
        if direct:
            nc.vector.tensor_add(out=dst_ap, in0=t1[:], in1=swp[:])
        else:
            nc.vector.tensor_add(out=t1[:], in0=t1[:], in1=swp[:])
            nc.sync.dma_start(dst_ap, t1[:])

    def kv_proj(t, xts):
        b, tloc = divmod(t * TCH, T)
        lsl = slice(tloc, tloc + TCH)
        ps = ps_proj.tile([128, 512], F32, tag="big", name="ps")[:, :TCH]
        for dc in range(DC):
            nc.tensor.matmul(ps[:], wk_sb[:, dc, :], xts[dc // 8][:, dc % 8, :],
                             start=(dc == 0), stop=(dc == DC - 1))
        rope_evict(ps, lsl, kts_sb[b][:, lsl], direct=True)

        ps = ps_proj.tile([128, 512], F32, tag="big", name="ps")[:, :TCH]
        for dc in range(DC):
            nc.tensor.matmul(ps[:], wv_sb[:, dc, :], xts[dc // 8][:, dc % 8, :],
                             start=(dc == 0), stop=(dc == DC - 1))
        vraw = rpool.tile([128, TCH], BF16, tag="vraw")
        nc.any.tensor_copy(vraw[:], ps[:])
        for j in range(TCH // 128):
            pst = ps_t.tile([128, 128], BF16, tag="pst", name="pst")
            nc.tensor.transpose(pst[:], vraw[:, j * 128:(j + 1) * 128],
                                ident_bf[:])
            g = (tloc // 128) + j
            nc.any.tensor_copy(vts_sb[b][:, g, :], pst[:])

    def q_proj(t, xts):
        b, tloc = divmod(t * TCH, T)
        lsl = slice(tloc, tloc + TCH)
        for fc in range(QH):
            ps = ps_proj.tile([128, 512], F32, tag="big", name="ps")[:, :TCH]
            for dc in range(DC):
                nc.tensor.matmul(
                    ps[:], wq_sb[fc][:, dc, :], xts[dc // 8][:, dc % 8, :],
                    start=(dc == 0), stop=(dc == DC - 1))
            rope_evict(ps, lsl, qT_s[b][fc][:, lsl])

    # phase-2 block order and qT fetch, defined here so the first two qT
    # blocks can be prefetched from inside the phase-1 loop (their DMA
    # issues then sit early in the scalar queue instead of behind all of
    # phase 1)
    blocks = [(b, qb, h) for b in range(B) for qb in range(NQB)
              for h in range(QH)]

    def fetch_qT(i):
        b, qb, h = blocks[i]
        qT_sb = qpool.tile([128, QB], BF16, tag="qT", name="qT_sb")
        nc.scalar.dma_start(qT_sb[:], qT_s[b][h][:, qb * QB:(qb + 1) * QB])
        return qT_sb

    qT_ring = []

    kv_proj(0, cur_x)
    q_proj(0, cur_x)
    cur_x = next_x
    for t in range(1, NTC):
        xts = cur_x
        if t + 1 < NTC:
            cur_x = fetch_x(t + 1)
        kv_proj(t, xts)
        q_proj(t, xts)
        if t == 4:
            # wo preload on the (otherwise idle) gpsimd SWDGE queue; delayed
            # past startup so it doesn't steal DMA bandwidth from x/weights
            for fc in range(QH):
                nc.gpsimd.dma_start(wo_sb[:, fc, :], wo[:, fc, :])
        if t == NTC - 3:
            qT_ring.append(fetch_qT(0))
            qT_ring.append(fetch_qT(1))

    rpool.release()
    cspool.release()
    xpool.release()
    wpool.release()
    ps_t.release()
    ps_proj.release()

    apool = tc.alloc_tile_pool(name="attn", bufs=4)
    opool = tc.alloc_tile_pool(name="outev", bufs=6)
    # PSUM rings (8 banks total): scores 2x[128,2,512] (4 banks), attnV
    # accumulators 2x (2 banks), denominator/wo-accumulator shared ring 2x
    # (2 banks). Keeping ps_yt (block-lived) OUT of the fast-cycling rings
    # avoids phase-3 matmuls landing on a bank still held by a live
    # attention accumulator.
    ps2pool = tc.alloc_tile_pool(name="ps2", bufs=2, space="PSUM")
    psy = tc.alloc_tile_pool(name="psy", bufs=2, space="PSUM")
    psm = tc.alloc_tile_pool(name="psm", bufs=2, space="PSUM")

    # ---- phase 2+3 interleaved ----
    # block order (b, qb, h): after the 4 heads of (b, qb) finish, the wo
    # matmuls for those 512 tokens are emitted (lag-1 behind the next
    # block's first score pair).
    pending = []     # deferred consume/tail thunks (lag-1 across blocks)
    pending_p3 = []  # deferred wo-matmul groups (drained one pair later so
                     # the previous group's DVE tail has PE work to hide
                     # behind before the wo matmuls need its yt tiles)
    yt_group = {}    # (b, qb) -> [yt tiles by h]

    def drain_pending():
        while pending:
            pending.pop(0)()

    def drain_pending_p3(n=None):
        cnt = 0
        while pending_p3 and (n is None or cnt < n):
            pending_p3.pop(0)()
            cnt += 1

    def phase3_tcl(b, tg, tcl, yts, last=False):
        row0 = b * T + tg * 512 + tcl * 128
        if True:
            # one full-row [128, 4096] out tile per 128 tokens: the single
            # 8KB-row DMA wins arbitration and issue-rate over 4 small ones
            ot = opool.tile([128, D], BF16, tag="ot", name="ot")
            for oc in range(D // 512):
                ps = psm.tile([128, 512], F32, tag="sm", name="pso")
                for fc in range(QH):
                    nc.tensor.matmul(
                        ps[:],
                        yts[fc][:, tcl * 128:(tcl + 1) * 128],
                        wo_sb[:, fc, oc * 512:(oc + 1) * 512],
                        start=(fc == 0), stop=(fc == QH - 1))
                # explicit engine alternation: nc.any tends to dump all
                # copies on ScalarE, whose FIFO then blocks exp
                if oc % 2 == 0:
                    nc.vector.tensor_copy(ot[:, oc * 512:(oc + 1) * 512],
                                          ps[:])
                else:
                    nc.scalar.copy(ot[:, oc * 512:(oc + 1) * 512], ps[:])
            nsplit = 4 if (last and tcl == 3) else 1
            for p in range(nsplit):
                r0 = row0 + p * 128 // nsplit
                r1 = row0 + (p + 1) * 128 // nsplit
                nc.sync.dma_start(out[r0:r1, :], ot[r0 - row0:r1 - row0, :])

    for i, (b, qb, h) in enumerate(blocks):
        nkc = 4 * (qb + 1)
        npair = nkc // 2
        qT_sb = qT_ring.pop(0)
        if i + 2 < len(blocks):
            qT_ring.append(fetch_qT(i + 2))

        ps_yt = psy.tile([128, 512], F32, tag="yt", name="ps_yt")
        yt = ypool.tile([128, QB], BF16, tag=f"yt{h}", name=f"yt{h}")
        yt_group.setdefault((b, qb), [None] * QH)[h] = yt
        a2s = []
        acc = [None]

        def consume(ip, nkc=nkc, qb=qb, ps_yt=ps_yt, a2s=a2s, vts=vts_sb[b]):
            a2 = a2s[ip]
            for j in range(2):
                c = 2 * ip + j
                qs = max(0, (c - 4 * qb) * 128)
                nc.tensor.matmul(ps_yt[:, qs:], vts[:, c, :], a2[:, j, qs:],
                                 start=(c == 0), stop=(c == nkc - 1))

        def ones_tail(b=b, qb=qb, h=h, ps_yt=ps_yt, acc=acc, yt=yt):
            ps_bc = psm.tile([128, 512], F32, tag="sm", name="ps_bc")
            nc.tensor.matmul(ps_bc[:], onesPP[:], acc[0][:],
                             start=True, stop=True)
            rb = qpool.tile([128, QB], F32, tag="rb", name="rb")
            nc.vector.reciprocal_approx_fast(out=rb[:], in_=ps_bc[:])
            nc.vector.tensor_mul(out=yt[:], in0=ps_yt[:], in1=rb[:])

        for ip in range(npair):
            # qsp: first query column this PAIR can influence; the score
            # matmul/exp/mask/pair-sum all restrict to [qsp:] so exp never
            # reads unwritten PSUM (masked-out garbage within [qsp:] is
            # zeroed by the BIG mask as in the unrestricted scheme)
            qsp = max(0, (2 * ip - 4 * qb) * 128)
            ps2 = ps2pool.tile([128, 2, 512], F32, tag="s", name="ps2")
            for j in range(2):
                c = 2 * ip + j
                nc.tensor.matmul(ps2[:, j, qsp:], kts_sb[b][:, c * 128:(c + 1) * 128],
                                 qT_sb[:, qsp:], start=True, stop=True)
            if ip == 0:
                # previous block's deferred tail runs behind our first
                # score pair, so its exp wait never idles the PE
                drain_pending()
            elif ip % 2 == 1:
                # one wo-matmul tcl-slice (32 MMs, ~7us of PE work) after
                # every other score pair: the reservoir hides the
                # exp+mask latency chain of the surrounding pairs
                # (drain capacity 80 points >= 64 slices)
                drain_pending_p3(1)
            a2 = apool.tile([128, 2, 512], BF16, tag="a", name="a2")
            nc.scalar.activation(a2[:, :, qsp:], ps2[:, :, qsp:], EXP,
                                 scale=SCALE)
            for j in range(2):
                c = 2 * ip + j
                delta = c * 128 - qb * QB
                if delta >= 0:
                    off = 384 - delta
                    nc.vector.tensor_mul(
                        out=a2[:, j, qsp:], in0=a2[:, j, qsp:],
                        in1=BIG[:, off + qsp:off + QB])
            if ip == 0:
                acc[0] = apool.tile([128, 512], BF16, tag="acc", name="acc")
                nc.vector.tensor_add(out=acc[0][:], in0=a2[:, 0, :],
                                     in1=a2[:, 1, :])
            else:
                tmp = apool.tile([128, 512], BF16, tag="tmp", name="tmp")
                nc.vector.tensor_add(out=tmp[:, qsp:], in0=a2[:, 0, qsp:],
                                     in1=a2[:, 1, qsp:])
                nc.vector.tensor_add(out=acc[0][:, qsp:],
                                     in0=acc[0][:, qsp:],
                                     in1=tmp[:, qsp:])
            a2s.append(a2)
            if ip >= 2:
                consume(ip - 2)
        if npair >= 2:
            pending.append(lambda c=consume, n=npair: c(n - 2))
        pending.append(lambda c=consume, n=npair: c(n - 1))
        pending.append(ones_tail)
        if h == QH - 1:
            last = (i == len(blocks) - 1)
            yts = yt_group.pop((b, qb))
            for tcl in range(4):
                pending_p3.append(
                    lambda b=b, qb=qb, tcl=tcl, yts=yts, last=last:
                    phase3_tcl(b, qb, tcl, yts, last=last))
            if last:
                drain_pending()
                drain_pending_p3()
    drain_pending()
    drain_pending_p3()

    opool.release()
    apool.release()
    psm.release()
    psy.release()
    ps2pool.release()
    qpool.release()
    ypool.release()
    kvpool.release()
    wopool.release()
    const.release()
    dram.release()


_PERM = np.concatenate([np.arange(0, HD, 2), np.arange(1, HD, 2)])


def _prep_inputs(x, freqs_cis, wq, wk, wv, wo):
    x = np.asarray(x, dtype=np.float32)
    freqs_cis = np.asarray(freqs_cis, dtype=np.float32)
    wq = np.asarray(wq, dtype=np.float32)
    wk = np.asarray(wk, dtype=np.float32)
    wv = np.asarray(wv, dtype=np.float32)
    wo = np.asarray(wo, dtype=np.float32)

    x2 = x.reshape(BT, D)
    # [di, tchunk, dc, tlocal] so each phase-1 chunk DMA is 128 x 32KB contig
    xTq = np.ascontiguousarray(
        x2.reshape(NTC, TCH, DC, 128).transpose(3, 0, 2, 1)).astype(BF)

    cosv = freqs_cis[:, :, 0].T                      # [64, T]
    sinv = freqs_cis[:, :, 1].T
    cc = np.ascontiguousarray(
        np.concatenate([cosv, cosv], axis=0)).astype(BF)   # [128, T]
    ss = np.ascontiguousarray(
        np.concatenate([-sinv, sinv], axis=0)).astype(BF)

    in_maps = []
    for c in range(N_CORES):
        wq_fc = np.stack([
            np.ascontiguousarray(
                wq[:, (4 * c + fc) * HD + _PERM]
                .reshape(DC, 128, 128).transpose(1, 0, 2))
            for fc in range(QH)])
        kcols = c * HD + _PERM
        in_maps.append({
            "xT": xTq,
            "wq": wq_fc.astype(BF),
            "wk": np.ascontiguousarray(
                wk[:, kcols].reshape(DC, 128, KF).transpose(1, 0, 2))
                .astype(BF),
            "wv": np.ascontiguousarray(
                wv[:, c * HD:(c + 1) * HD].reshape(DC, 128, KF)
                .transpose(1, 0, 2)).astype(BF),
            "wo": np.ascontiguousarray(
                wo[c * QF:(c + 1) * QF, :].reshape(QH, 128, D)
                .transpose(1, 0, 2)).astype(BF),
            "cc": cc,
            "ss": ss,
        })
    return in_maps


def kernel(x, freqs_cis, wq, wk, wv, wo):
    if not _NC_CACHE:
        _NC_CACHE.append(build())
    nc = _NC_CACHE[0]
    in_maps = _prep_inputs(x, freqs_cis, wq, wk, wv, wo)
    res = None
    err = None
    for _attempt in range(3):
        try:
            res = bass_utils.run_bass_kernel_spmd(
                nc, in_maps, core_ids=list(range(N_CORES)))
            break
        except Exception as e:  # transient NRT device wedge: retry
            err = e
            import time as _time
            _time.sleep(5)
    if res is None:
        raise err
    acc = res.results[0]["out"].astype(np.float32)
    for i in range(1, N_CORES):
        acc += res.results[i]["out"].astype(np.float32)
    return acc.reshape(B, T, D)


if __name__ == "__main__":
    rng = np.random.default_rng(0)
    s = 1.0 / np.sqrt(D)
    inputs = {
        "x": rng.standard_normal((B, T, D), dtype=np.float32),
        "freqs_cis": rng.standard_normal((T, HD // 2, 2), dtype=np.float32),
        "wq": rng.standard_normal((D, NH * HD), dtype=np.float32) * s,
        "wk": rng.standard_normal((D, NKV * HD), dtype=np.float32) * s,
        "wv": rng.standard_normal((D, NKV * HD), dtype=np.float32) * s,
        "wo": rng.standard_normal((D, D), dtype=np.float32) * s,
    }
    out = kernel(**inputs)
    print("out", out.shape, out.dtype, float(np.abs(out).mean()))
